# revision 23
# baseline (speedup 1.0000x reference)
"""AttenComm (gnn_message_passing) Trainium2 kernel.

Pipeline per scene b (B=4, ragged N_b = record_len[b] CAVs):
  1. bilinear-warp each CAV feature map (C,H,W) into ego frame (affine from
     pairwise_t_matrix[b,0,j])
  2. per-pixel scaled-dot-product attention, ego row only:
       s_j(p) = <f_0(p), f_j(p)>/sqrt(C);  a = softmax_j(s);  out = sum_j a_j f_j

Sharding: 8 cores = (scene b, h-half).  Host precomputes per-pixel bilinear
tap indices + weights (all border cases folded into the weights); x is
host-transposed to pixel-major bf16 [H*W, C].  Device (SPMD, uniform across
cores): indirect DMA pulls the 4 tap channel-vectors per output pixel into
[pixel-partition, channel-free] tiles; the 4-tap weighted sum and the
attention-weighted output sum run on the tensor engine via diagonal weight
matmuls (diag built with one tensor_scalar on a constant identity);
scores use fused scalar_tensor_tensor accum; softmax = reduce_max + Exp
(bias=-max, accum_out=denominator) + reciprocal; output normalization is
folded into the PSUM->SBUF evacuation (activation Copy with per-partition
scale).
"""

import math
import os
import sys

import numpy as np

for _p in ("/opt/trn_rl_repo", "/opt/pypackages"):
    if _p not in sys.path:
        sys.path.insert(0, _p)

import ml_dtypes

B, L = 4, 5
C, H, W = 256, 128, 256
HW = H * W
NS = 5              # uniform node slots per core (SPMD)
HALF = H // 2       # rows per shard
NPX = HALF * W      # output pixels per shard = 16384
NBLK = NPX // 128   # 128-pixel blocks per shard = 128
GB = 4              # blocks per gather group
NGRP = NBLK // GB   # 32 groups
DOWNSAMPLE_RATE = 4.0
DISCRETE_RATIO = 0.4
INV_SQRT_C = 1.0 / math.sqrt(C)
BIGNEG = -1.0e30
PAD_ROWS = 128      # extra zero rows after HW in xt so reads at idx HW stay in-bounds


# ---------------------------------------------------------------------------
# Host-side: affine grids, tap indices, weights
# ---------------------------------------------------------------------------

def _norm_affines(pairwise_t_matrix):
    """(B,L,L,4,4) -> (B,L,2,3) normalized affines for to-ego warps (row 0)."""
    t = pairwise_t_matrix[:, :, :, (0, 1), :][:, :, :, :, (0, 1, 3)].astype(np.float64)
    t = t.copy()
    t[..., 0, 1] *= H / W
    t[..., 1, 0] *= W / H
    t[..., 0, 2] *= 2.0 / (DOWNSAMPLE_RATE * DISCRETE_RATIO * W)
    t[..., 1, 2] *= 2.0 / (DOWNSAMPLE_RATE * DISCRETE_RATIO * H)
    return t[:, 0]  # (B, L, 2, 3): transform for node j of scene b


def _taps_for(M2, h_lo, h_hi):
    """Bilinear taps for output rows [h_lo, h_hi) under 2x3 affine M2.

    Returns base0, base1 (int32 pixel-row indices into the [HW, C] image;
    the x0 tap is at base, the x0+1 tap at base+1) and w (npx, 4) float32
    tap weights (order: y0x0, y0x1, y1x0, y1x1) with border handling folded
    in.
    """
    ys = np.linspace(-1.0, 1.0, H)
    xs = np.linspace(-1.0, 1.0, W)
    gy, gx = np.meshgrid(ys[h_lo:h_hi], xs, indexing="ij")  # (hh, W)
    cx = M2[0, 0] * gx + M2[0, 1] * gy + M2[0, 2]
    cy = M2[1, 0] * gx + M2[1, 1] * gy + M2[1, 2]
    px = (cx + 1.0) * 0.5 * (W - 1)
    py = (cy + 1.0) * 0.5 * (H - 1)
    x0 = np.floor(px)
    y0 = np.floor(py)
    wx = (px - x0).astype(np.float64)
    wy = (py - y0).astype(np.float64)
    x0 = x0.astype(np.int64)
    y0 = y0.astype(np.int64)
    x1, y1 = x0 + 1, y0 + 1

    vx0 = (x0 >= 0) & (x0 <= W - 1)
    vx1 = (x1 >= 0) & (x1 <= W - 1)
    vy0 = (y0 >= 0) & (y0 <= H - 1)
    vy1 = (y1 >= 0) & (y1 <= H - 1)

    w00 = (1 - wx) * (1 - wy) * vx0 * vy0
    w01 = wx * (1 - wy) * vx1 * vy0
    w10 = (1 - wx) * wy * vx0 * vy1
    w11 = wx * wy * vx1 * vy1

    def pair_base(yr, vyr, wa, wb):
        base = yr * W + x0
        wa = wa.copy()
        wb = wb.copy()
        dead = ~vyr
        wa[dead] = 0.0
        wb[dead] = 0.0
        base = np.where(vyr, base, 0)
        # base == -1 happens only for yr==0, x0==-1: shift right one pixel;
        # slot a then holds pixel (0,0) which is the x1 tap -> move wb to wa.
        neg = base < 0
        base = np.where(neg, 0, base)
        wa = np.where(neg, wb, wa)
        wb = np.where(neg, 0.0, wb)
        return base, wa, wb

    b0, w00, w01 = pair_base(y0, vy0, w00, w01)
    b1, w10, w11 = pair_base(y1, vy1, w10, w11)
    w = np.stack([w00, w01, w10, w11], axis=-1).reshape(-1, 4).astype(np.float32)
    return b0.reshape(-1).astype(np.int32), b1.reshape(-1).astype(np.int32), w


def _build_core_aux(M2_list, n_real):
    """Aux arrays for one core: goff (pair-gather offsets for warped slots),
    gw (tap weights), bneg (dummy-slot mask).

    goff: [128, (NS-1)*NBLK*2] int32; col ((jj-1)*NBLK + t)*2 + r = partition
    p's pixel-row index for block t, slot jj, y-pair r.  The gather pulls 512
    contiguous channels = pixel rows (idx, idx+1) = the (x0, x0+1) taps.
    """
    goff = np.zeros((128, (NS - 1) * NBLK * 2), dtype=np.int32)
    gw = np.zeros((128, NS * NBLK * 4), dtype=np.float32)
    bneg = np.zeros((128, NS), dtype=np.float32)
    for jj in range(NS):
        if jj >= n_real:
            bneg[:, jj] = BIGNEG
            continue
        if jj == 0:
            continue  # ego slot: identity warp, no gather offsets/weights
        b0, b1, w = M2_list[jj]
        for t in range(NBLK):
            sl = slice(t * 128, (t + 1) * 128)
            for tap in range(4):
                gw[:, (jj * NBLK + t) * 4 + tap] = w[sl, tap]
            goff[:, ((jj - 1) * NBLK + t) * 2 + 0] = b0[sl]
            goff[:, ((jj - 1) * NBLK + t) * 2 + 1] = b1[sl]
    return goff, gw, bneg


# ---------------------------------------------------------------------------
# Device kernel (bass / Tile)
# ---------------------------------------------------------------------------

def _build_nc():
    import concourse.bass as bass
    import concourse.bacc as bacc
    import concourse.mybir as mybir
    from concourse.tile import TileContext

    fp32 = mybir.dt.float32
    bf16 = mybir.dt.bfloat16
    i32 = mybir.dt.int32
    Alu = mybir.AluOpType
    Act = mybir.ActivationFunctionType
    axis_x = mybir.AxisListType.X

    nc = bacc.Bacc("TRN2")
    xt = [
        nc.dram_tensor(f"xt{jj}", [HW + PAD_ROWS, C], bf16, kind="ExternalInput")
        for jj in range(NS)
    ]
    goff_d = nc.dram_tensor(
        "goff", [128, (NS - 1) * NBLK * 2], i32, kind="ExternalInput"
    )
    xe = nc.dram_tensor("xe", [NPX, C], bf16, kind="ExternalInput")
    gw_d = nc.dram_tensor("gw", [128, NS * NBLK * 4], fp32, kind="ExternalInput")
    bneg_d = nc.dram_tensor("bneg", [128, NS], fp32, kind="ExternalInput")
    ident_d = nc.dram_tensor("ident", [128, 128], bf16, kind="ExternalInput")
    outp = nc.dram_tensor("outp", [NPX, C], fp32, kind="ExternalOutput")

    with TileContext(nc) as tc:
        with (
            tc.tile_pool(name="const", bufs=1) as const_pool,
            tc.tile_pool(name="gat", bufs=4) as gat_pool,
            tc.tile_pool(name="ego", bufs=4) as ego_pool,
            tc.tile_pool(name="diag", bufs=12) as diag_pool,
            tc.tile_pool(name="feat", bufs=3) as feat_pool,
            tc.tile_pool(name="attn", bufs=6) as attn_pool,
            tc.tile_pool(name="outs", bufs=4) as out_pool,
            tc.tile_pool(name="psf", bufs=4, space="PSUM") as psf_pool,
            tc.tile_pool(name="pso", bufs=3, space="PSUM") as pso_pool,
        ):
            goff_sb = const_pool.tile([128, (NS - 1) * NBLK * 2], i32)
            nc.sync.dma_start(out=goff_sb, in_=goff_d[:, :])
            gw_sb = const_pool.tile([128, NS * NBLK * 4], fp32)
            nc.sync.dma_start(out=gw_sb, in_=gw_d[:, :])
            bneg_sb = const_pool.tile([128, NS], fp32)
            nc.sync.dma_start(out=bneg_sb, in_=bneg_d[:, :])
            ident_sb = const_pool.tile([128, 128], bf16)
            nc.sync.dma_start(out=ident_sb, in_=ident_d[:, :])

            for t in range(NBLK):
                f0 = ego_pool.tile([128, C], bf16, tag="f0")
                nc.sync.dma_start(out=f0, in_=xe[t * 128:(t + 1) * 128, :])
                gys = {}
                for jj in range(1, NS):
                    for r in range(2):
                        gy = gat_pool.tile([128, 512], bf16, tag=f"gy{jj}r{r}")
                        col = ((jj - 1) * NBLK + t) * 2 + r
                        nc.gpsimd.indirect_dma_start(
                            out=gy,
                            out_offset=None,
                            in_=bass.AP(xt[jj], 0, [[C, HW + PAD_ROWS], [1, C]]),
                            in_offset=bass.IndirectOffsetOnAxis(
                                ap=goff_sb[:, col:col + 1], axis=0
                            ),
                        )
                        gys[(jj, r)] = gy

                feats = [f0]
                s5 = attn_pool.tile([128, NS], fp32, tag="s5")
                scr = attn_pool.tile([128, C], bf16, tag="scr")
                nc.vector.scalar_tensor_tensor(
                    out=scr, in0=f0, scalar=INV_SQRT_C, in1=f0,
                    op0=Alu.mult, op1=Alu.mult, accum_out=s5[:, 0:1],
                )
                for jj in range(1, NS):
                    psf = psf_pool.tile([128, C], fp32, tag="psf")
                    for tap in range(4):
                        col = (jj * NBLK + t) * 4 + tap
                        d = diag_pool.tile([128, 128], bf16, tag="D")
                        nc.vector.tensor_scalar_mul(
                            d, ident_sb, gw_sb[:, col:col + 1]
                        )
                        gy = gys[(jj, tap // 2)]
                        half = (tap % 2) * C
                        nc.tensor.matmul(
                            psf, d, gy[:, half:half + C],
                            start=(tap == 0), stop=(tap == 3),
                        )
                    f = feat_pool.tile([128, C], bf16, tag=f"f{jj}")
                    nc.scalar.activation(f, psf, Act.Copy)
                    feats.append(f)
                    nc.vector.scalar_tensor_tensor(
                        out=scr, in0=f, scalar=INV_SQRT_C, in1=f0,
                        op0=Alu.mult, op1=Alu.mult, accum_out=s5[:, jj:jj + 1],
                    )
                # mask dummy slots, softmax over slots
                nc.vector.tensor_tensor(s5, s5, bneg_sb, op=Alu.add)
                nmax = attn_pool.tile([128, 1], fp32, tag="nmax")
                nc.vector.tensor_reduce(
                    nmax, s5, axis=axis_x, op=Alu.max, negate=True
                )
                e5 = attn_pool.tile([128, NS], fp32, tag="e5")
                den = attn_pool.tile([128, 1], fp32, tag="den")
                nc.scalar.activation(
                    e5, s5, Act.Exp, bias=nmax[:, 0:1], scale=1.0, accum_out=den
                )
                rden = attn_pool.tile([128, 1], fp32, tag="rden")
                nc.vector.reciprocal(rden, den)

                pso = pso_pool.tile([128, C], fp32, tag="pso")
                for jj in range(NS):
                    d = diag_pool.tile([128, 128], bf16, tag="D")
                    nc.vector.tensor_scalar_mul(d, ident_sb, e5[:, jj:jj + 1])
                    nc.tensor.matmul(
                        pso, d, feats[jj], start=(jj == 0), stop=(jj == NS - 1)
                    )
                ot = out_pool.tile([128, C], fp32, tag="ot")
                nc.scalar.activation(ot, pso, Act.Copy, scale=rden[:, 0:1])
                nc.sync.dma_start(out=outp[t * 128:(t + 1) * 128, :], in_=ot)
    nc.compile()
    return nc


# ---------------------------------------------------------------------------
# Host emulation of the exact device program (for correctness dev)
# ---------------------------------------------------------------------------

def _emulate_core(xt_list, xe, goff, gw, bneg):
    out = np.zeros((NPX, C), dtype=np.float32)
    xts = [a.astype(np.float32) for a in xt_list]
    xef = xe.astype(np.float32)
    for t in range(NBLK):
        f0 = xef[t * 128:(t + 1) * 128].astype(ml_dtypes.bfloat16)
        feats = [f0]
        s5 = np.zeros((128, NS), dtype=np.float32)
        prod = f0.astype(np.float32) * INV_SQRT_C * f0.astype(np.float32)
        s5[:, 0] = prod.sum(axis=1)
        for jj in range(1, NS):
            psf = np.zeros((128, C), dtype=np.float32)
            for tap in range(4):
                col = (jj * NBLK + t) * 4 + tap
                wv = gw[:, col].astype(ml_dtypes.bfloat16).astype(np.float32)
                gcol = ((jj - 1) * NBLK + t) * 2 + tap // 2
                idx = goff[:, gcol].astype(np.int64)
                flat = xts[jj].reshape(-1)
                start = idx * C + (tap % 2) * C
                gath = flat[start[:, None] + np.arange(C)[None, :]].astype(
                    ml_dtypes.bfloat16
                )
                psf += wv[:, None] * gath.astype(np.float32)
            f = psf.astype(ml_dtypes.bfloat16)
            feats.append(f)
            prod = f.astype(np.float32) * INV_SQRT_C * f0.astype(np.float32)
            s5[:, jj] = prod.sum(axis=1)
        s5 = s5 + bneg
        mx = s5.max(axis=1, keepdims=True)
        e5 = np.exp(s5 - mx)
        den = e5.sum(axis=1)
        pso = np.zeros((128, C), dtype=np.float32)
        for jj in range(NS):
            pso += e5[:, jj][:, None] * feats[jj].astype(np.float32)
        out[t * 128:(t + 1) * 128] = pso / den[:, None]
    return out


# ---------------------------------------------------------------------------
# Entry point
# ---------------------------------------------------------------------------

def kernel(x, rm, record_len, pairwise_t_matrix):
    x = np.asarray(x, dtype=np.float32)
    rl = np.asarray(record_len).astype(np.int64)
    M = _norm_affines(np.asarray(pairwise_t_matrix, dtype=np.float32))
    starts = np.concatenate([[0], np.cumsum(rl)[:-1]]).astype(np.int64)

    zero_xt = np.zeros((HW + PAD_ROWS, C), dtype=ml_dtypes.bfloat16)
    ident = np.eye(128, dtype=ml_dtypes.bfloat16)

    in_maps = []
    core_meta = []
    for core in range(8):
        b, half = core // 2, core % 2
        n_real = int(rl[b]) if b < len(rl) else 0
        n_real = min(n_real, NS)
        h_lo, h_hi = half * HALF, (half + 1) * HALF
        xt_list = []
        M2_list = []
        for jj in range(NS):
            if jj < n_real:
                node = x[int(starts[b]) + jj]  # (C, H, W)
                xtj = np.zeros((HW + PAD_ROWS, C), dtype=ml_dtypes.bfloat16)
                xtj[:HW] = node.reshape(C, HW).T.astype(ml_dtypes.bfloat16)
                xt_list.append(xtj)
                M2_list.append(_taps_for(M[b, jj], h_lo, h_hi))
            else:
                xt_list.append(zero_xt)
        goff, gw, bneg = _build_core_aux(M2_list, n_real)
        xe = np.ascontiguousarray(xt_list[0][h_lo * W:h_lo * W + NPX])
        im = {f"xt{jj}": xt_list[jj] for jj in range(NS)}
        im.update(goff=goff, gw=gw, bneg=bneg, ident=ident, xe=xe)
        in_maps.append(im)
        core_meta.append((b, half))

    if os.environ.get("BASS_SIM_HOST") == "1":
        outs = [
            _emulate_core(
                [im[f"xt{jj}"] for jj in range(NS)],
                im["xe"], im["goff"], im["gw"], im["bneg"],
            )
            for im in in_maps
        ]
    else:
        from concourse import bass_utils

        nc = _build_nc()
        trace = os.environ.get("KERNEL_TRACE") == "1"
        res = bass_utils.run_bass_kernel_spmd(
            nc, in_maps, core_ids=list(range(8)), trace=trace
        )
        outs = [res.results[i]["outp"] for i in range(8)]
        if trace:
            kernel.last_exec_time_ns = res.exec_time_ns

    out = np.zeros((B, C, H, W), dtype=np.float32)
    for core, (b, half) in enumerate(core_meta):
        o = np.asarray(outs[core], dtype=np.float32)  # (NPX, C)
        out[b, :, half * HALF:(half + 1) * HALF, :] = (
            o.reshape(HALF, W, C).transpose(2, 0, 1)
        )
    return out


kernel.last_exec_time_ns = None


# revision 30
# speedup vs baseline: 1.0119x; 1.0119x over previous
"""AttenComm (gnn_message_passing) Trainium2 kernel.

Pipeline per scene b (B=4, ragged N_b = record_len[b] CAVs):
  1. bilinear-warp each CAV feature map (C,H,W) into ego frame (affine from
     pairwise_t_matrix[b,0,j])
  2. per-pixel scaled-dot-product attention, ego row only:
       s_j(p) = <f_0(p), f_j(p)>/sqrt(C);  a = softmax_j(s);  out = sum_j a_j f_j

Sharding: 8 cores = (scene b, h-half).  Host precomputes per-pixel bilinear
tap indices + weights (all border cases folded into the weights); x is
host-transposed to pixel-major bf16 [H*W, C].  Device (SPMD, uniform across
cores): indirect DMA pulls the 4 tap channel-vectors per output pixel into
[pixel-partition, channel-free] tiles; the 4-tap weighted sum and the
attention-weighted output sum run on the tensor engine via diagonal weight
matmuls (diag built with one tensor_scalar on a constant identity);
scores use fused scalar_tensor_tensor accum; softmax = reduce_max + Exp
(bias=-max, accum_out=denominator) + reciprocal; output normalization is
folded into the PSUM->SBUF evacuation (activation Copy with per-partition
scale).
"""

import math
import os
import sys

import numpy as np

for _p in ("/opt/trn_rl_repo", "/opt/pypackages"):
    if _p not in sys.path:
        sys.path.insert(0, _p)

import ml_dtypes

B, L = 4, 5
C, H, W = 256, 128, 256
HW = H * W
NS = 5              # uniform node slots per core (SPMD)
HALF = H // 2       # rows per shard
NPX = HALF * W      # output pixels per shard = 16384
NBLK = NPX // 128   # 128-pixel blocks per shard = 128
GB = 4              # blocks per gather group
NGRP = NBLK // GB   # 32 groups
DOWNSAMPLE_RATE = 4.0
DISCRETE_RATIO = 0.4
INV_SQRT_C = 1.0 / math.sqrt(C)
BIGNEG = -1.0e30
PAD_ROWS = 128      # extra zero rows after HW in xt so reads at idx HW stay in-bounds


# ---------------------------------------------------------------------------
# Host-side: affine grids, tap indices, weights
# ---------------------------------------------------------------------------

def _norm_affines(pairwise_t_matrix):
    """(B,L,L,4,4) -> (B,L,2,3) normalized affines for to-ego warps (row 0)."""
    t = pairwise_t_matrix[:, :, :, (0, 1), :][:, :, :, :, (0, 1, 3)].astype(np.float64)
    t = t.copy()
    t[..., 0, 1] *= H / W
    t[..., 1, 0] *= W / H
    t[..., 0, 2] *= 2.0 / (DOWNSAMPLE_RATE * DISCRETE_RATIO * W)
    t[..., 1, 2] *= 2.0 / (DOWNSAMPLE_RATE * DISCRETE_RATIO * H)
    return t[:, 0]  # (B, L, 2, 3): transform for node j of scene b


def _taps_for(M2, h_lo, h_hi):
    """Bilinear taps for output rows [h_lo, h_hi) under 2x3 affine M2.

    Returns base0, base1 (int32 pixel-row indices into the [HW, C] image;
    the x0 tap is at base, the x0+1 tap at base+1) and w (npx, 4) float32
    tap weights (order: y0x0, y0x1, y1x0, y1x1) with border handling folded
    in.
    """
    ys = np.linspace(-1.0, 1.0, H)
    xs = np.linspace(-1.0, 1.0, W)
    gy, gx = np.meshgrid(ys[h_lo:h_hi], xs, indexing="ij")  # (hh, W)
    cx = M2[0, 0] * gx + M2[0, 1] * gy + M2[0, 2]
    cy = M2[1, 0] * gx + M2[1, 1] * gy + M2[1, 2]
    px = (cx + 1.0) * 0.5 * (W - 1)
    py = (cy + 1.0) * 0.5 * (H - 1)
    x0 = np.floor(px)
    y0 = np.floor(py)
    wx = (px - x0).astype(np.float64)
    wy = (py - y0).astype(np.float64)
    x0 = x0.astype(np.int64)
    y0 = y0.astype(np.int64)
    x1, y1 = x0 + 1, y0 + 1

    vx0 = (x0 >= 0) & (x0 <= W - 1)
    vx1 = (x1 >= 0) & (x1 <= W - 1)
    vy0 = (y0 >= 0) & (y0 <= H - 1)
    vy1 = (y1 >= 0) & (y1 <= H - 1)

    w00 = (1 - wx) * (1 - wy) * vx0 * vy0
    w01 = wx * (1 - wy) * vx1 * vy0
    w10 = (1 - wx) * wy * vx0 * vy1
    w11 = wx * wy * vx1 * vy1

    def pair_base(yr, vyr, wa, wb):
        base = yr * W + x0
        wa = wa.copy()
        wb = wb.copy()
        dead = ~vyr
        wa[dead] = 0.0
        wb[dead] = 0.0
        base = np.where(vyr, base, 0)
        # base == -1 happens only for yr==0, x0==-1: shift right one pixel;
        # slot a then holds pixel (0,0) which is the x1 tap -> move wb to wa.
        neg = base < 0
        base = np.where(neg, 0, base)
        wa = np.where(neg, wb, wa)
        wb = np.where(neg, 0.0, wb)
        return base, wa, wb

    b0, w00, w01 = pair_base(y0, vy0, w00, w01)
    b1, w10, w11 = pair_base(y1, vy1, w10, w11)
    w = np.stack([w00, w01, w10, w11], axis=-1).reshape(-1, 4).astype(np.float32)
    return b0.reshape(-1).astype(np.int32), b1.reshape(-1).astype(np.int32), w


def _build_core_aux(M2_list, n_real):
    """Aux for one core: goff (shared row-pair gather bases), gw, bneg.

    One gather per (warped slot, gather-index gi in 0..NBLK+1): gi < NBLK is
    block gi's y0-pair; gi = NBLK/NBLK+1 are the virtual row-64 pairs (the
    y1-pairs of the last two blocks).  Block t's y1 taps read gather t+2,
    which normally holds exactly (y1, x0..x0+1); where floor-crossings make
    it differ by +-1 pixel the host remaps the tap weights to the slots the
    gather actually holds and drops the (provably tiny) unmatched weight.
    """
    NG = NBLK + 2
    goff = np.zeros((128, (NS - 1) * NG), dtype=np.int32)
    gw = np.zeros((128, NS * NBLK * 4), dtype=np.float32)
    bneg = np.zeros((128, NS), dtype=np.float32)
    for jj in range(NS):
        if jj >= n_real:
            bneg[:, jj] = BIGNEG
            continue
        if jj == 0:
            continue
        b0, b1, w = M2_list[jj]
        base = (jj - 1) * NG
        for t in range(NBLK):
            goff[:, base + t] = b0[t * 128:(t + 1) * 128]
        goff[:, base + NBLK] = b1[(NBLK - 2) * 128:(NBLK - 1) * 128]
        goff[:, base + NBLK + 1] = b1[(NBLK - 1) * 128:NBLK * 128]
        for t in range(NBLK):
            sl = slice(t * 128, (t + 1) * 128)
            col = (jj * NBLK + t) * 4
            gw[:, col + 0] = w[sl, 0]
            gw[:, col + 1] = w[sl, 1]
            desired = b1[sl]                       # pixel id of the w10 tap
            if t + 2 < NBLK:
                held = b0[(t + 2) * 128:(t + 3) * 128]
            else:
                held = desired                     # virtual gathers match exactly
            w10v, w11v = w[sl, 2], w[sl, 3]
            gw[:, col + 2] = np.where(
                held == desired, w10v, np.where(held == desired + 1, w11v, 0.0)
            )
            gw[:, col + 3] = np.where(
                held == desired, w11v, np.where(held == desired - 1, w10v, 0.0)
            )
    return goff, gw, bneg


# ---------------------------------------------------------------------------
# Device kernel (bass / Tile)
# ---------------------------------------------------------------------------

def _build_nc():
    import concourse.bass as bass
    import concourse.bacc as bacc
    import concourse.mybir as mybir
    from concourse.tile import TileContext

    fp32 = mybir.dt.float32
    bf16 = mybir.dt.bfloat16
    i32 = mybir.dt.int32
    Alu = mybir.AluOpType
    Act = mybir.ActivationFunctionType
    axis_x = mybir.AxisListType.X

    nc = bacc.Bacc("TRN2")
    xt = [
        nc.dram_tensor(f"xt{jj}", [HW + PAD_ROWS, C], bf16, kind="ExternalInput")
        for jj in range(NS)
    ]
    goff_d = nc.dram_tensor(
        "goff", [128, (NS - 1) * (NBLK + 2)], i32, kind="ExternalInput"
    )
    xe = nc.dram_tensor("xe", [NPX, C], bf16, kind="ExternalInput")
    gw_d = nc.dram_tensor("gw", [128, NS * NBLK * 4], fp32, kind="ExternalInput")
    bneg_d = nc.dram_tensor("bneg", [128, NS], fp32, kind="ExternalInput")
    ident_d = nc.dram_tensor("ident", [128, 128], bf16, kind="ExternalInput")
    outp = nc.dram_tensor("outp", [NPX, C], fp32, kind="ExternalOutput")

    with TileContext(nc) as tc:
        with (
            tc.tile_pool(name="const", bufs=1) as const_pool,
            tc.tile_pool(name="gat", bufs=8) as gat_pool,
            tc.tile_pool(name="ego", bufs=6) as ego_pool,
            tc.tile_pool(name="diag", bufs=16) as diag_pool,
            tc.tile_pool(name="feat", bufs=5) as feat_pool,
            tc.tile_pool(name="attn", bufs=10) as attn_pool,
            tc.tile_pool(name="outs", bufs=6) as out_pool,
            tc.tile_pool(name="psf", bufs=5, space="PSUM") as psf_pool,
            tc.tile_pool(name="pso", bufs=3, space="PSUM") as pso_pool,
        ):
            goff_sb = const_pool.tile([128, (NS - 1) * (NBLK + 2)], i32)
            nc.sync.dma_start(out=goff_sb, in_=goff_d[:, :])
            gw_sb = const_pool.tile([128, NS * NBLK * 4], fp32)
            nc.sync.dma_start(out=gw_sb, in_=gw_d[:, :])
            bneg_sb = const_pool.tile([128, NS], fp32)
            nc.sync.dma_start(out=bneg_sb, in_=bneg_d[:, :])
            ident_sb = const_pool.tile([128, 128], bf16)
            nc.sync.dma_start(out=ident_sb, in_=ident_d[:, :])

            NG = NBLK + 2
            gring = {jj: [] for jj in range(1, NS)}
            for gi in range(NG):
                for jj in range(1, NS):
                    gy = gat_pool.tile([128, 512], bf16, tag=f"gy{jj}")
                    col = (jj - 1) * NG + gi
                    nc.gpsimd.indirect_dma_start(
                        out=gy,
                        out_offset=None,
                        in_=bass.AP(xt[jj], 0, [[C, HW + PAD_ROWS - 1], [1, C]]),
                        in_offset=bass.IndirectOffsetOnAxis(
                            ap=goff_sb[:, col:col + 1], axis=0
                        ),
                    )
                    gring[jj].append(gy)
                if gi < 2:
                    continue
                t = gi - 2
                f0 = ego_pool.tile([128, C], bf16, tag="f0")
                nc.sync.dma_start(out=f0, in_=xe[t * 128:(t + 1) * 128, :])
                feats = [f0]
                s5 = attn_pool.tile([128, NS], fp32, tag="s5")
                scr = attn_pool.tile([128, C], bf16, tag="scr")
                nc.vector.scalar_tensor_tensor(
                    out=scr, in0=f0, scalar=INV_SQRT_C, in1=f0,
                    op0=Alu.mult, op1=Alu.mult, accum_out=s5[:, 0:1],
                )
                for jj in range(1, NS):
                    psf = psf_pool.tile([128, C], fp32, tag="psf")
                    for tap in range(4):
                        col = (jj * NBLK + t) * 4 + tap
                        d = diag_pool.tile([128, 128], bf16, tag="D")
                        if tap == 1 and jj in (1, 3):
                            wb = gw_sb[:, col:col + 1].to_broadcast([128, 128])
                            nc.gpsimd.tensor_tensor(d, ident_sb, wb, op=Alu.mult)
                        else:
                            nc.vector.tensor_scalar_mul(
                                d, ident_sb, gw_sb[:, col:col + 1]
                            )
                        gy = gring[jj][t if tap < 2 else t + 2]
                        half = (tap % 2) * C
                        nc.tensor.matmul(
                            psf, d, gy[:, half:half + C],
                            start=(tap == 0), stop=(tap == 3),
                        )
                    f = feat_pool.tile([128, C], bf16, tag=f"f{jj}")
                    nc.scalar.activation(f, psf, Act.Copy)
                    feats.append(f)
                    nc.vector.scalar_tensor_tensor(
                        out=scr, in0=f, scalar=INV_SQRT_C, in1=f0,
                        op0=Alu.mult, op1=Alu.mult, accum_out=s5[:, jj:jj + 1],
                    )
                # mask dummy slots, softmax over slots
                nc.vector.tensor_tensor(s5, s5, bneg_sb, op=Alu.add)
                nmax = attn_pool.tile([128, 1], fp32, tag="nmax")
                nc.vector.tensor_reduce(
                    nmax, s5, axis=axis_x, op=Alu.max, negate=True
                )
                e5 = attn_pool.tile([128, NS], fp32, tag="e5")
                den = attn_pool.tile([128, 1], fp32, tag="den")
                nc.scalar.activation(
                    e5, s5, Act.Exp, bias=nmax[:, 0:1], scale=1.0, accum_out=den
                )
                rden = attn_pool.tile([128, 1], fp32, tag="rden")
                nc.vector.reciprocal(rden, den)

                pso = pso_pool.tile([128, C], fp32, tag="pso")
                for jj in range(NS):
                    d = diag_pool.tile([128, 128], bf16, tag="D")
                    nc.vector.tensor_scalar_mul(d, ident_sb, e5[:, jj:jj + 1])
                    nc.tensor.matmul(
                        pso, d, feats[jj], start=(jj == 0), stop=(jj == NS - 1)
                    )
                ot = out_pool.tile([128, C], fp32, tag="ot")
                nc.scalar.activation(ot, pso, Act.Copy, scale=rden[:, 0:1])
                nc.sync.dma_start(out=outp[t * 128:(t + 1) * 128, :], in_=ot)
    nc.compile()
    return nc


# ---------------------------------------------------------------------------
# Host emulation of the exact device program (for correctness dev)
# ---------------------------------------------------------------------------

def _emulate_core(xt_list, xe, goff, gw, bneg):
    out = np.zeros((NPX, C), dtype=np.float32)
    xts = [a.astype(np.float32) for a in xt_list]
    xef = xe.astype(np.float32)
    NG = NBLK + 2
    gring = {}
    for jj in range(1, NS):
        flat = xts[jj].reshape(-1)
        tiles = []
        for gi in range(NG):
            idx = goff[:, (jj - 1) * NG + gi].astype(np.int64)
            start = idx * C
            tiles.append(
                flat[start[:, None] + np.arange(512)[None, :]].astype(
                    ml_dtypes.bfloat16
                )
            )
        gring[jj] = tiles
    for t in range(NBLK):
        f0 = xef[t * 128:(t + 1) * 128].astype(ml_dtypes.bfloat16)
        feats = [f0]
        s5 = np.zeros((128, NS), dtype=np.float32)
        prod = f0.astype(np.float32) * INV_SQRT_C * f0.astype(np.float32)
        s5[:, 0] = prod.sum(axis=1)
        for jj in range(1, NS):
            psf = np.zeros((128, C), dtype=np.float32)
            for tap in range(4):
                col = (jj * NBLK + t) * 4 + tap
                wv = gw[:, col].astype(ml_dtypes.bfloat16).astype(np.float32)
                gy = gring[jj][t if tap < 2 else t + 2]
                half = (tap % 2) * C
                psf += wv[:, None] * gy[:, half:half + C].astype(np.float32)
            f = psf.astype(ml_dtypes.bfloat16)
            feats.append(f)
            prod = f.astype(np.float32) * INV_SQRT_C * f0.astype(np.float32)
            s5[:, jj] = prod.sum(axis=1)
        s5 = s5 + bneg
        mx = s5.max(axis=1, keepdims=True)
        e5 = np.exp(s5 - mx)
        den = e5.sum(axis=1)
        pso = np.zeros((128, C), dtype=np.float32)
        for jj in range(NS):
            pso += e5[:, jj][:, None] * feats[jj].astype(np.float32)
        out[t * 128:(t + 1) * 128] = pso / den[:, None]
    return out


# ---------------------------------------------------------------------------
# Entry point
# ---------------------------------------------------------------------------

def kernel(x, rm, record_len, pairwise_t_matrix):
    x = np.asarray(x, dtype=np.float32)
    rl = np.asarray(record_len).astype(np.int64)
    M = _norm_affines(np.asarray(pairwise_t_matrix, dtype=np.float32))
    starts = np.concatenate([[0], np.cumsum(rl)[:-1]]).astype(np.int64)

    zero_xt = np.zeros((HW + PAD_ROWS, C), dtype=ml_dtypes.bfloat16)
    ident = np.eye(128, dtype=ml_dtypes.bfloat16)

    in_maps = []
    core_meta = []
    for core in range(8):
        b, half = core // 2, core % 2
        n_real = int(rl[b]) if b < len(rl) else 0
        n_real = min(n_real, NS)
        h_lo, h_hi = half * HALF, (half + 1) * HALF
        xt_list = []
        M2_list = []
        for jj in range(NS):
            if jj < n_real:
                node = x[int(starts[b]) + jj]  # (C, H, W)
                xtj = np.zeros((HW + PAD_ROWS, C), dtype=ml_dtypes.bfloat16)
                xtj[:HW] = node.reshape(C, HW).T.astype(ml_dtypes.bfloat16)
                xt_list.append(xtj)
                M2_list.append(_taps_for(M[b, jj], h_lo, h_hi))
            else:
                xt_list.append(zero_xt)
        goff, gw, bneg = _build_core_aux(M2_list, n_real)
        xe = np.ascontiguousarray(xt_list[0][h_lo * W:h_lo * W + NPX])
        im = {f"xt{jj}": xt_list[jj] for jj in range(NS)}
        im.update(goff=goff, gw=gw, bneg=bneg, ident=ident, xe=xe)
        in_maps.append(im)
        core_meta.append((b, half))

    if os.environ.get("BASS_SIM_HOST") == "1":
        outs = [
            _emulate_core(
                [im[f"xt{jj}"] for jj in range(NS)],
                im["xe"], im["goff"], im["gw"], im["bneg"],
            )
            for im in in_maps
        ]
    else:
        from concourse import bass_utils

        nc = _build_nc()
        trace = os.environ.get("KERNEL_TRACE") == "1"
        res = bass_utils.run_bass_kernel_spmd(
            nc, in_maps, core_ids=list(range(8)), trace=trace
        )
        outs = [res.results[i]["outp"] for i in range(8)]
        if trace:
            kernel.last_exec_time_ns = res.exec_time_ns

    out = np.zeros((B, C, H, W), dtype=np.float32)
    for core, (b, half) in enumerate(core_meta):
        o = np.asarray(outs[core], dtype=np.float32)  # (NPX, C)
        out[b, :, half * HALF:(half + 1) * HALF, :] = (
            o.reshape(HALF, W, C).transpose(2, 0, 1)
        )
    return out


kernel.last_exec_time_ns = None


# revision 35
# speedup vs baseline: 1.0596x; 1.0471x over previous
"""AttenComm (gnn_message_passing) Trainium2 kernel.

Pipeline per scene b (B=4, ragged N_b = record_len[b] CAVs):
  1. bilinear-warp each CAV feature map (C,H,W) into ego frame (affine from
     pairwise_t_matrix[b,0,j])
  2. per-pixel scaled-dot-product attention, ego row only:
       s_j(p) = <f_0(p), f_j(p)>/sqrt(C);  a = softmax_j(s);  out = sum_j a_j f_j

Sharding: 8 cores = (scene b, h-half).  Host precomputes per-pixel bilinear
tap indices + weights (all border cases folded into the weights); x is
host-transposed to pixel-major bf16 [H*W, C].  Device (SPMD, uniform across
cores): indirect DMA pulls the 4 tap channel-vectors per output pixel into
[pixel-partition, channel-free] tiles; the 4-tap weighted sum and the
attention-weighted output sum run on the tensor engine via diagonal weight
matmuls (diag built with one tensor_scalar on a constant identity);
scores use fused scalar_tensor_tensor accum; softmax = reduce_max + Exp
(bias=-max, accum_out=denominator) + reciprocal; output normalization is
folded into the PSUM->SBUF evacuation (activation Copy with per-partition
scale).
"""

import math
import os
import sys

import numpy as np

for _p in ("/opt/trn_rl_repo", "/opt/pypackages"):
    if _p not in sys.path:
        sys.path.insert(0, _p)

import ml_dtypes

B, L = 4, 5
C, H, W = 256, 128, 256
HW = H * W
NS = 5              # uniform node slots per core (SPMD)
HALF = H // 2       # rows per shard
NPX = HALF * W      # output pixels per shard = 16384
NBLK = NPX // 128   # 128-pixel blocks per shard = 128
GB = 4              # blocks per gather group
NGRP = NBLK // GB   # 32 groups
DOWNSAMPLE_RATE = 4.0
DISCRETE_RATIO = 0.4
INV_SQRT_C = 1.0 / math.sqrt(C)
BIGNEG = -1.0e30
PAD_ROWS = 128      # extra zero rows after HW in xt so reads at idx HW stay in-bounds


# ---------------------------------------------------------------------------
# Host-side: affine grids, tap indices, weights
# ---------------------------------------------------------------------------

def _norm_affines(pairwise_t_matrix):
    """(B,L,L,4,4) -> (B,L,2,3) normalized affines for to-ego warps (row 0)."""
    t = pairwise_t_matrix[:, :, :, (0, 1), :][:, :, :, :, (0, 1, 3)].astype(np.float64)
    t = t.copy()
    t[..., 0, 1] *= H / W
    t[..., 1, 0] *= W / H
    t[..., 0, 2] *= 2.0 / (DOWNSAMPLE_RATE * DISCRETE_RATIO * W)
    t[..., 1, 2] *= 2.0 / (DOWNSAMPLE_RATE * DISCRETE_RATIO * H)
    return t[:, 0]  # (B, L, 2, 3): transform for node j of scene b


def _taps_for(M2, h_lo, h_hi):
    """Bilinear taps for output rows [h_lo, h_hi) under 2x3 affine M2.

    Returns base0, base1 (int32 pixel-row indices into the [HW, C] image;
    the x0 tap is at base, the x0+1 tap at base+1) and w (npx, 4) float32
    tap weights (order: y0x0, y0x1, y1x0, y1x1) with border handling folded
    in.
    """
    ys = np.linspace(-1.0, 1.0, H)
    xs = np.linspace(-1.0, 1.0, W)
    gy, gx = np.meshgrid(ys[h_lo:h_hi], xs, indexing="ij")  # (hh, W)
    cx = M2[0, 0] * gx + M2[0, 1] * gy + M2[0, 2]
    cy = M2[1, 0] * gx + M2[1, 1] * gy + M2[1, 2]
    px = (cx + 1.0) * 0.5 * (W - 1)
    py = (cy + 1.0) * 0.5 * (H - 1)
    x0 = np.floor(px)
    y0 = np.floor(py)
    wx = (px - x0).astype(np.float64)
    wy = (py - y0).astype(np.float64)
    x0 = x0.astype(np.int64)
    y0 = y0.astype(np.int64)
    x1, y1 = x0 + 1, y0 + 1

    vx0 = (x0 >= 0) & (x0 <= W - 1)
    vx1 = (x1 >= 0) & (x1 <= W - 1)
    vy0 = (y0 >= 0) & (y0 <= H - 1)
    vy1 = (y1 >= 0) & (y1 <= H - 1)

    w00 = (1 - wx) * (1 - wy) * vx0 * vy0
    w01 = wx * (1 - wy) * vx1 * vy0
    w10 = (1 - wx) * wy * vx0 * vy1
    w11 = wx * wy * vx1 * vy1

    def pair_base(yr, vyr, wa, wb):
        base = yr * W + x0
        wa = wa.copy()
        wb = wb.copy()
        dead = ~vyr
        wa[dead] = 0.0
        wb[dead] = 0.0
        base = np.where(vyr, base, 0)
        # base == -1 happens only for yr==0, x0==-1: shift right one pixel;
        # slot a then holds pixel (0,0) which is the x1 tap -> move wb to wa.
        neg = base < 0
        base = np.where(neg, 0, base)
        wa = np.where(neg, wb, wa)
        wb = np.where(neg, 0.0, wb)
        return base, wa, wb

    b0, w00, w01 = pair_base(y0, vy0, w00, w01)
    b1, w10, w11 = pair_base(y1, vy1, w10, w11)
    w = np.stack([w00, w01, w10, w11], axis=-1).reshape(-1, 4).astype(np.float32)
    return b0.reshape(-1).astype(np.int32), b1.reshape(-1).astype(np.int32), w


def _build_core_aux(M2_list, n_real):
    """Aux for one core: goff (shared row-pair gather bases), gw, bneg.

    One gather per (warped slot, gather-index gi in 0..NBLK+1): gi < NBLK is
    block gi's y0-pair; gi = NBLK/NBLK+1 are the virtual row-64 pairs (the
    y1-pairs of the last two blocks).  Block t's y1 taps read gather t+2,
    which normally holds exactly (y1, x0..x0+1); where floor-crossings make
    it differ by +-1 pixel the host remaps the tap weights to the slots the
    gather actually holds and drops the (provably tiny) unmatched weight.
    """
    NG = NBLK + 2
    goff = np.zeros((128, (NS - 1) * NG), dtype=np.int32)
    gw = np.zeros((128, NS * NBLK * 4), dtype=np.float32)
    bneg = np.zeros((128, NS), dtype=np.float32)
    for jj in range(NS):
        if jj >= n_real:
            bneg[:, jj] = BIGNEG
            continue
        if jj == 0:
            continue
        b0, b1, w = M2_list[jj]
        base = (jj - 1) * NG
        for t in range(NBLK):
            goff[:, base + t] = b0[t * 128:(t + 1) * 128]
        goff[:, base + NBLK] = b1[(NBLK - 2) * 128:(NBLK - 1) * 128]
        goff[:, base + NBLK + 1] = b1[(NBLK - 1) * 128:NBLK * 128]
        for t in range(NBLK):
            sl = slice(t * 128, (t + 1) * 128)
            col = (jj * NBLK + t) * 4
            gw[:, col + 0] = w[sl, 0]
            gw[:, col + 1] = w[sl, 1]
            desired = b1[sl]                       # pixel id of the w10 tap
            if t + 2 < NBLK:
                held = b0[(t + 2) * 128:(t + 3) * 128]
            else:
                held = desired                     # virtual gathers match exactly
            w10v, w11v = w[sl, 2], w[sl, 3]
            gw[:, col + 2] = np.where(
                held == desired, w10v, np.where(held == desired + 1, w11v, 0.0)
            )
            gw[:, col + 3] = np.where(
                held == desired, w11v, np.where(held == desired - 1, w10v, 0.0)
            )
    return goff, gw, bneg


# ---------------------------------------------------------------------------
# Device kernel (bass / Tile)
# ---------------------------------------------------------------------------

def _build_nc():
    import concourse.bass as bass
    import concourse.bacc as bacc
    import concourse.mybir as mybir
    from concourse.tile import TileContext

    fp32 = mybir.dt.float32
    bf16 = mybir.dt.bfloat16
    i32 = mybir.dt.int32
    Alu = mybir.AluOpType
    Act = mybir.ActivationFunctionType
    axis_x = mybir.AxisListType.X

    nc = bacc.Bacc("TRN2")
    xt = [
        nc.dram_tensor(f"xt{jj}", [HW + PAD_ROWS, C], bf16, kind="ExternalInput")
        for jj in range(NS)
    ]
    goff_d = nc.dram_tensor(
        "goff", [128, (NS - 1) * (NBLK + 2)], i32, kind="ExternalInput"
    )
    xe = nc.dram_tensor("xe", [NPX, C], bf16, kind="ExternalInput")
    gw_d = nc.dram_tensor("gw", [128, NS * NBLK * 4], fp32, kind="ExternalInput")
    bneg_d = nc.dram_tensor("bneg", [128, NS], fp32, kind="ExternalInput")
    ident_d = nc.dram_tensor("ident", [128, 128], bf16, kind="ExternalInput")
    outp = nc.dram_tensor("outp", [NPX, C], fp32, kind="ExternalOutput")

    with TileContext(nc) as tc:
        with (
            tc.tile_pool(name="const", bufs=1) as const_pool,
            tc.tile_pool(name="gat", bufs=8) as gat_pool,
            tc.tile_pool(name="ego", bufs=6) as ego_pool,
            tc.tile_pool(name="diag", bufs=16) as diag_pool,
            tc.tile_pool(name="feat", bufs=5) as feat_pool,
            tc.tile_pool(name="attn", bufs=10) as attn_pool,
            tc.tile_pool(name="outs", bufs=6) as out_pool,
            tc.tile_pool(name="psf", bufs=5, space="PSUM") as psf_pool,
            tc.tile_pool(name="pso", bufs=3, space="PSUM") as pso_pool,
        ):
            goff_sb = const_pool.tile([128, (NS - 1) * (NBLK + 2)], i32)
            nc.sync.dma_start(out=goff_sb, in_=goff_d[:, :])
            gw_sb = const_pool.tile([128, NS * NBLK * 4], fp32)
            nc.sync.dma_start(out=gw_sb, in_=gw_d[:, :])
            bneg_sb = const_pool.tile([128, NS], fp32)
            nc.sync.dma_start(out=bneg_sb, in_=bneg_d[:, :])
            ident_sb = const_pool.tile([128, 128], bf16)
            nc.sync.dma_start(out=ident_sb, in_=ident_d[:, :])

            NG = NBLK + 2
            gring = {jj: [] for jj in range(1, NS)}
            for gi in range(NG):
                for jj in range(1, NS):
                    gy = gat_pool.tile([128, 512], bf16, tag=f"gy{jj}")
                    col = (jj - 1) * NG + gi
                    nc.gpsimd.indirect_dma_start(
                        out=gy,
                        out_offset=None,
                        in_=bass.AP(xt[jj], 0, [[C, HW + PAD_ROWS - 1], [1, C]]),
                        in_offset=bass.IndirectOffsetOnAxis(
                            ap=goff_sb[:, col:col + 1], axis=0
                        ),
                    )
                    gring[jj].append(gy)
                if gi < 2:
                    continue
                t = gi - 2
                f0 = ego_pool.tile([128, C], bf16, tag="f0")
                nc.sync.dma_start(out=f0, in_=xe[t * 128:(t + 1) * 128, :])
                feats = [f0]
                s5 = attn_pool.tile([128, NS], fp32, tag="s5")
                scr = attn_pool.tile([128, C], bf16, tag="scr")
                nc.vector.scalar_tensor_tensor(
                    out=scr, in0=f0, scalar=INV_SQRT_C, in1=f0,
                    op0=Alu.mult, op1=Alu.mult, accum_out=s5[:, 0:1],
                )
                for jj in range(1, NS):
                    psf = psf_pool.tile([128, C], fp32, tag="psf")
                    for tap in range(4):
                        col = (jj * NBLK + t) * 4 + tap
                        d = diag_pool.tile([128, 128], bf16, tag="D")
                        if (tap == 1 and jj in (1, 2, 3)) or (tap == 3 and jj == 2):
                            wb = gw_sb[:, col:col + 1].to_broadcast([128, 128])
                            nc.gpsimd.tensor_tensor(d, ident_sb, wb, op=Alu.mult)
                        else:
                            nc.vector.tensor_scalar_mul(
                                d, ident_sb, gw_sb[:, col:col + 1]
                            )
                        gy = gring[jj][t if tap < 2 else t + 2]
                        half = (tap % 2) * C
                        nc.tensor.matmul(
                            psf, d, gy[:, half:half + C],
                            start=(tap == 0), stop=(tap == 3),
                        )
                    f = feat_pool.tile([128, C], bf16, tag=f"f{jj}")
                    nc.scalar.activation(f, psf, Act.Copy)
                    feats.append(f)
                    nc.vector.scalar_tensor_tensor(
                        out=scr, in0=f, scalar=INV_SQRT_C, in1=f0,
                        op0=Alu.mult, op1=Alu.mult, accum_out=s5[:, jj:jj + 1],
                    )
                # mask dummy slots, softmax over slots
                nc.vector.tensor_tensor(s5, s5, bneg_sb, op=Alu.add)
                nmax = attn_pool.tile([128, 1], fp32, tag="nmax")
                nc.vector.tensor_reduce(
                    nmax, s5, axis=axis_x, op=Alu.max, negate=True
                )
                e5 = attn_pool.tile([128, NS], fp32, tag="e5")
                den = attn_pool.tile([128, 1], fp32, tag="den")
                nc.scalar.activation(
                    e5, s5, Act.Exp, bias=nmax[:, 0:1], scale=1.0, accum_out=den
                )
                rden = attn_pool.tile([128, 1], fp32, tag="rden")
                nc.vector.reciprocal(rden, den)

                pso = pso_pool.tile([128, C], fp32, tag="pso")
                for jj in range(NS):
                    d = diag_pool.tile([128, 128], bf16, tag="D")
                    nc.vector.tensor_scalar_mul(d, ident_sb, e5[:, jj:jj + 1])
                    nc.tensor.matmul(
                        pso, d, feats[jj], start=(jj == 0), stop=(jj == NS - 1)
                    )
                ot = out_pool.tile([128, C], fp32, tag="ot")
                nc.scalar.activation(ot, pso, Act.Copy, scale=rden[:, 0:1])
                nc.sync.dma_start(out=outp[t * 128:(t + 1) * 128, :], in_=ot)
    nc.compile()
    return nc


# ---------------------------------------------------------------------------
# Host emulation of the exact device program (for correctness dev)
# ---------------------------------------------------------------------------

def _emulate_core(xt_list, xe, goff, gw, bneg):
    out = np.zeros((NPX, C), dtype=np.float32)
    xts = [a.astype(np.float32) for a in xt_list]
    xef = xe.astype(np.float32)
    NG = NBLK + 2
    gring = {}
    for jj in range(1, NS):
        flat = xts[jj].reshape(-1)
        tiles = []
        for gi in range(NG):
            idx = goff[:, (jj - 1) * NG + gi].astype(np.int64)
            start = idx * C
            tiles.append(
                flat[start[:, None] + np.arange(512)[None, :]].astype(
                    ml_dtypes.bfloat16
                )
            )
        gring[jj] = tiles
    for t in range(NBLK):
        f0 = xef[t * 128:(t + 1) * 128].astype(ml_dtypes.bfloat16)
        feats = [f0]
        s5 = np.zeros((128, NS), dtype=np.float32)
        prod = f0.astype(np.float32) * INV_SQRT_C * f0.astype(np.float32)
        s5[:, 0] = prod.sum(axis=1)
        for jj in range(1, NS):
            psf = np.zeros((128, C), dtype=np.float32)
            for tap in range(4):
                col = (jj * NBLK + t) * 4 + tap
                wv = gw[:, col].astype(ml_dtypes.bfloat16).astype(np.float32)
                gy = gring[jj][t if tap < 2 else t + 2]
                half = (tap % 2) * C
                psf += wv[:, None] * gy[:, half:half + C].astype(np.float32)
            f = psf.astype(ml_dtypes.bfloat16)
            feats.append(f)
            prod = f.astype(np.float32) * INV_SQRT_C * f0.astype(np.float32)
            s5[:, jj] = prod.sum(axis=1)
        s5 = s5 + bneg
        mx = s5.max(axis=1, keepdims=True)
        e5 = np.exp(s5 - mx)
        den = e5.sum(axis=1)
        pso = np.zeros((128, C), dtype=np.float32)
        for jj in range(NS):
            pso += e5[:, jj][:, None] * feats[jj].astype(np.float32)
        out[t * 128:(t + 1) * 128] = pso / den[:, None]
    return out


# ---------------------------------------------------------------------------
# Entry point
# ---------------------------------------------------------------------------

def kernel(x, rm, record_len, pairwise_t_matrix):
    x = np.asarray(x, dtype=np.float32)
    rl = np.asarray(record_len).astype(np.int64)
    M = _norm_affines(np.asarray(pairwise_t_matrix, dtype=np.float32))
    starts = np.concatenate([[0], np.cumsum(rl)[:-1]]).astype(np.int64)

    zero_xt = np.zeros((HW + PAD_ROWS, C), dtype=ml_dtypes.bfloat16)
    ident = np.eye(128, dtype=ml_dtypes.bfloat16)

    in_maps = []
    core_meta = []
    for core in range(8):
        b, half = core // 2, core % 2
        n_real = int(rl[b]) if b < len(rl) else 0
        n_real = min(n_real, NS)
        h_lo, h_hi = half * HALF, (half + 1) * HALF
        xt_list = []
        M2_list = []
        for jj in range(NS):
            if jj < n_real:
                node = x[int(starts[b]) + jj]  # (C, H, W)
                xtj = np.zeros((HW + PAD_ROWS, C), dtype=ml_dtypes.bfloat16)
                xtj[:HW] = node.reshape(C, HW).T.astype(ml_dtypes.bfloat16)
                xt_list.append(xtj)
                M2_list.append(_taps_for(M[b, jj], h_lo, h_hi))
            else:
                xt_list.append(zero_xt)
        goff, gw, bneg = _build_core_aux(M2_list, n_real)
        xe = np.ascontiguousarray(xt_list[0][h_lo * W:h_lo * W + NPX])
        im = {f"xt{jj}": xt_list[jj] for jj in range(NS)}
        im.update(goff=goff, gw=gw, bneg=bneg, ident=ident, xe=xe)
        in_maps.append(im)
        core_meta.append((b, half))

    if os.environ.get("BASS_SIM_HOST") == "1":
        outs = [
            _emulate_core(
                [im[f"xt{jj}"] for jj in range(NS)],
                im["xe"], im["goff"], im["gw"], im["bneg"],
            )
            for im in in_maps
        ]
    else:
        from concourse import bass_utils

        nc = _build_nc()
        trace = os.environ.get("KERNEL_TRACE") == "1"
        res = bass_utils.run_bass_kernel_spmd(
            nc, in_maps, core_ids=list(range(8)), trace=trace
        )
        outs = [res.results[i]["outp"] for i in range(8)]
        if trace:
            kernel.last_exec_time_ns = res.exec_time_ns

    out = np.zeros((B, C, H, W), dtype=np.float32)
    for core, (b, half) in enumerate(core_meta):
        o = np.asarray(outs[core], dtype=np.float32)  # (NPX, C)
        out[b, :, half * HALF:(half + 1) * HALF, :] = (
            o.reshape(HALF, W, C).transpose(2, 0, 1)
        )
    return out


kernel.last_exec_time_ns = None
